# revision 41
# baseline (speedup 1.0000x reference)
# Trainium2 Bass kernel for nn_AttnModel_64098091926054.
#
# Strategy: pure data parallel over batch (256 boards -> 32 per core x 8 cores),
# with the 32 per-core boards further split into two software-pipelined halves
# of 16 so the two dependence chains interleave across PE/DVE/ACT and hide the
# cross-engine semaphore latency.
#
# Design notes:
#   - All GEMMs run "weights stationary": out^T = W^T x lands directly in the
#     x^T layout [128 dout-part, chunk*16+board], so the residual stream never
#     needs PE transposes. Each half's x is kept 4x-replicated chunk-major
#     ([128, 4kt*4c*16]); a 0-stride AP view presents it as the 8-fold
#     replication the 128-partition attention layout needs.
#   - afin/fc0/fc1 ship as fp8 (e3m4) with per-tensor power-of-2 scales folded
#     into the activation scales. Validated: max rel err ~1e-3 vs f32
#     (the gate is 2e-2); verified on hardware.
#   - Attention per half runs on 128 partitions: 81 cells padded to 8 groups
#     of 11 (pads zeroed via the bfeat constants; a 19th ones-feature row
#     produces the softmax row-sums inside the same reduce). Offsets j=0/j=6
#     are duplicates and folded host-side (6 unique nb features).
#   - Biases enter PSUM via K=1 matmuls whose [1,128] rows live on partitions
#     {0,32,64} of a carrier tile (single-partition DMAs are pathologically
#     slow; matmul operands only allow base partitions 0/32/64).
#   - Layer 0's attention is constant-folded on the host (x==0), its unused
#     afin block is not even DMA'd, and the final log-softmax runs on the
#     host, so the device tail ends at the raw-logit matmul.
#   - All DVE attention traffic is bf16 (2x DVE throughput); the residual
#     stream is bf16.

import numpy as np
import ml_dtypes

import concourse.bass as bass
import concourse.bacc as bacc
import concourse.mybir as mybir
import concourse.tile as tile
from concourse.bass_utils import run_bass_kernel_spmd
from concourse.masks import make_identity

BS, D, L, B, P, POSD, J = 9, 512, 8, 256, 81, 12, 19
NCORES = 8
NB = B // NCORES            # 32 boards per core
HB = NB // 2                # 16 boards per pipelined half
HG, HPC = 8, 11             # 81 cells -> 8 partition-groups x 11 (7 pads)
JN = 6                      # unique neighbourhood offsets (j=6 dup of j=0)
QW = POSD + JN              # 18 g-projection features: pos12 | nb6
JF = JN + POSD + 1          # 19 s-side rows: nb6 | pos12 | ones (rowsum/cfin)
OFFSETS = [(-1, 0), (-1, 1), (0, -1), (0, 0), (0, 1), (-1, -1), (-1, 0)]

f32 = mybir.dt.float32
bf16 = mybir.dt.bfloat16
f8e3 = mybir.dt.float8e3
bf16_np = ml_dtypes.bfloat16
f8e3_np = ml_dtypes.float8_e3m4

# cpk (bf16, [128, CB_END]) columns: bft1 x2 | bfjp x2 | e2c8 | sT0 | wpost
BT1 = HPC * QW                    # 198
BJP = JF * HPC                    # 209
C_BT1 = 0                         # two half-blocks back to back
C_BJP = C_BT1 + 2 * BT1           # 396
C_E2C = C_BJP + 2 * BJP           # 814
C_E2T = C_E2C + HB                # 830: (16,128) replication mask
C_ST0 = C_E2T + 128               # 958
C_POST = C_ST0 + NB               # 990
CB_END = C_POST + P               # 1071

NROWS = L + L * 8                 # 8 qk_b rows + 64 fc bias rows
NRB = NROWS // 3                  # 24 row-blocks per carrier partition
WF8_F = 3 * 16 * 128              # 6144 cols: [afin|fc0|fc1] x (kt,m) tiles


def _positions():
    lin = np.linspace(0.0, 1.0, BS, dtype=np.float32)
    rs, cs = np.meshgrid(lin, lin, indexing="ij")
    zs = (rs + cs) / 2.0
    xs = np.stack([rs, cs, zs], -1).astype(np.float32)
    feats = []
    for p in [4.0 / (BS - 1), 16.0 / (BS - 1)]:
        a = (2.0 * np.pi * xs / p).astype(np.float32)
        feats.append(np.concatenate([np.cos(a), np.sin(a)], -1).astype(np.float32))
    return np.concatenate(feats, -1)  # (9, 9, 12)


def _prepare(obs, pos):
    single = obs[..., 0] - obs[..., 1]
    aug = np.pad(single, ((0, 0), (1, 1), (1, 1)))
    w = aug.shape[-1]
    outs = [aug[:, 1 + r : w - 1 + r, 1 + c : w - 1 + c] for (r, c) in OFFSETS]
    neigh = np.stack(outs, -1)
    n = obs.shape[0]
    stack = np.concatenate(
        [neigh, np.broadcast_to(pos, (n,) + pos.shape)], -1
    ).astype(np.float32)
    return stack.reshape(n, P, J)  # (B, 81, 19)


def _fold(inp):
    scale = np.float32(1.0 / np.sqrt(D))
    Wk = inp["kvb_w"][:, :, :D]
    Wv = inp["kvb_w"][:, :, D:]
    kvx_v = inp["kvx_w"][:, :, D:]
    qk_w = np.einsum("ldh,ljh->ldj", inp["q_w"], Wk) * scale      # (L,512,19)
    qk_b = np.einsum("lh,ljh->lj", inp["q_b"], Wk) * scale        # (L,19)
    afin = np.einsum("lde,leh->ldh", kvx_v, inp["fin_w"])         # (L,512,512)
    sfin = np.einsum("lje,leh->ljh", Wv, inp["fin_w"])            # (L,19,512)
    bias_v = inp["kvx_b"][:, D:] + inp["kvb_b"][:, D:]
    cfin = np.einsum("le,leh->lh", bias_v, inp["fin_w"]) + inp["fin_b"]
    return qk_w, qk_b, afin, sfin, cfin


def _quant_e3(w):
    s = np.float32(2.0 ** np.floor(np.log2(14.0 / np.abs(w).max())))
    return (w * s).astype(f8e3_np), s


def bcast_mid(ap2d, n):
    # (p, k) AP -> (p, n, k) with step-0 broadcast in the middle
    return bass.AP(
        tensor=ap2d.tensor, offset=ap2d.offset,
        ap=[ap2d.ap[0], [0, n], ap2d.ap[1]],
    )


def _build_nc(alphas, scales):
    """alphas: per-layer float; scales: (sa, s0, s1) per layer."""
    nc = bacc.Bacc("TRN2", target_bir_lowering=False, debug=False)

    d_cpk = nc.dram_tensor("cpk", [128, CB_END], bf16, kind="ExternalInput")
    d_sfal = nc.dram_tensor("sfal", [JF, L * D], bf16, kind="ExternalInput")
    d_wqk = nc.dram_tensor("wqk", [128, L * 4 * QW], bf16, kind="ExternalInput")
    d_wrows = nc.dram_tensor("wrows", [3, NRB * 128], bf16, kind="ExternalInput")
    d_whead = nc.dram_tensor("whead", [128, 4 * POSD], bf16, kind="ExternalInput")
    d_wf8 = nc.dram_tensor("wf8", [L, 128, WF8_F], f8e3, kind="ExternalInput")
    d_out = nc.dram_tensor("out", [NB, P], f32, kind="ExternalOutput")

    AX = mybir.AxisListType.X
    MUL = mybir.AluOpType.mult
    ADD = mybir.AluOpType.add

    with tile.TileContext(nc) as tc, nc.allow_low_precision(
        reason="bf16 attention path validated host-side: ~1e-3 vs 2e-2 gate"
    ):
        with (
            tc.tile_pool(name="consts", bufs=1) as consts,
            tc.tile_pool(name="wpool", bufs=8) as wpool,
            tc.tile_pool(name="ap", bufs=3) as apool,
            tc.tile_pool(name="pm", bufs=4, space="PSUM") as pm,
            tc.tile_pool(name="pt", bufs=3, space="PSUM") as pt,
        ):
            # ---- constants; DMA order tuned so layer 0 starts ASAP ----
            cpk = consts.tile([128, CB_END], bf16)
            nc.sync.dma_start(out=cpk, in_=d_cpk[:, :])
            sfal = consts.tile([JF, L * D], bf16)
            nc.sync.dma_start(out=sfal[:, 0:D], in_=d_sfal[:, 0:D])
            wf8l = [
                wpool.tile([128, WF8_F], f8e3, tag="wb", name=f"wb{i}")
                for i in range(L)
            ]
            # layer 0 never reads its afin block (x==0): ship fc0/fc1 only
            nc.sync.dma_start(out=wf8l[0][:, 2048:], in_=d_wf8[0, :, 2048:])
            # bias rows spread over partitions {0,32,64}
            wrow4 = consts.tile([65, NRB * 128], bf16)
            nc.sync.dma_start(out=wrow4[0:65:32, :], in_=d_wrows[:, :])
            wqk = consts.tile([128, L * 4 * QW], bf16)
            nc.sync.dma_start(out=wqk, in_=d_wqk[:, :])
            for l in range(1, L):
                nc.sync.dma_start(
                    out=sfal[:, l * D : (l + 1) * D], in_=d_sfal[:, l * D : (l + 1) * D]
                )
            whead = consts.tile([128, 4 * POSD], bf16)
            nc.sync.dma_start(out=whead, in_=d_whead[:, :])
            for l in range(1, L):
                nc.sync.dma_start(out=wf8l[l], in_=d_wf8[l, :, :])

            onesr4 = consts.tile([65, 128], bf16)
            nc.vector.memset(onesr4, 1.0)
            ident = consts.tile([HB, HB], bf16)
            make_identity(nc, ident[:, :])

            bft1 = [
                cpk[:, C_BT1 + h * BT1 : C_BT1 + (h + 1) * BT1].rearrange(
                    "p (a b) -> p a b", b=QW
                )
                for h in range(2)
            ]
            bfjp = [
                cpk[:, C_BJP + h * BJP : C_BJP + (h + 1) * BJP].rearrange(
                    "p (a b) -> p a b", b=HPC
                )
                for h in range(2)
            ]
            e2c8 = cpk[:, C_E2C:C_E2T]          # (128, 16) group-sum mask
            e2c8T = cpk[:HB, C_E2T:C_E2T + 128]  # (16, 128) replication mask
            sT0 = cpk[:JF, C_ST0:C_POST]        # (19, 32) layer-0 attention
            wpost = cpk[:POSD, C_POST:CB_END]   # (12, 81)

            def _row(r, ncols=128):
                p3 = 32 * (r % 3)
                i = r // 3
                return wrow4[p3 : p3 + 1, i * 128 : i * 128 + ncols]

            def ones_at(r, n):
                p3 = 32 * (r % 3)
                return onesr4[p3 : p3 + 1, :n]

            def fcb_row(l, stage, m):
                return _row(L + l * 8 + stage * 4 + m)

            # per-half residual stream: plain bf16 [128, 4kt*16b]
            xch = []
            for h in range(2):
                t = apool.tile([128, 4 * HB], bf16, tag=f"xc{h}", name=f"xc_{h}")
                nc.vector.memset(t, 0.0)
                xch.append(t)

            def xbat(h, kt, n=HB):
                return xch[h][:, kt * HB : kt * HB + n]

            def residual_update(h, br_sb, positive):
                nxc = apool.tile([128, 4 * HB], bf16, tag=f"xc{h}",
                                 name=f"nxc_{h}")
                op = ADD if positive else mybir.AluOpType.subtract
                nc.vector.tensor_tensor(nxc, xch[h], br_sb, op=op)
                xch[h] = nxc

            def make_xrep8(h):
                # 8-fold replicated copy for the attention projections:
                # out col 128*kt + 16*c8 + b reads x[b] of chunk kt
                # (0-stride input APs are fine on DVE, unlike matmuls)
                r8 = apool.tile([128, 4 * 128], bf16, tag=f"xr{h}",
                                name=f"xr8_{h}")
                old = xch[h]
                in_ap = bass.AP(tensor=old.tensor, offset=old.offset,
                                ap=[old.ap[0], [HB, 4], [0, HG], [1, HB]])
                out_ap = bass.AP(tensor=r8.tensor, offset=r8.offset,
                                 ap=[r8.ap[0], [128, 4], [HB, HG], [1, HB]])
                nc.vector.tensor_copy(out_ap, in_ap)
                return r8

            for l in range(L):
                wb = wf8l[l]
                sa, s0, s1 = scales[l]
                al = alphas[l]

                # ---- attention (both halves, interleaved) ----
                if l == 0:
                    sT = [sT0[:, 0:HB], sT0[:, HB:NB]]
                else:
                    blk = (l * 4) * QW
                    g3p, g3, t1, dots, e4, t2, s4r = ({} for _ in range(7))
                    grp, recip, s_sb, sTp = {}, {}, {}, {}
                    sT = {}
                    xr8 = {}
                    for h in range(2):
                        xr8[h] = make_xrep8(h)
                    for h in range(2):
                        g3p[h] = pt.tile([128, QW], f32, tag="tp",
                                         name=f"g3p{h}")
                        for kt in range(4):
                            nc.tensor.matmul(
                                g3p[h], xr8[h][:, kt * 128 : (kt + 1) * 128],
                                wqk[:, blk + kt * QW : blk + (kt + 1) * QW],
                                start=(kt == 0), stop=False,
                            )
                        nc.tensor.matmul(
                            g3p[h], ones_at(l, 128), _row(l, QW),
                            start=False, stop=True,
                        )
                    for h in range(2):
                        g3[h] = apool.tile([128, QW], bf16, tag="g3",
                                           name=f"g3{h}")
                        nc.scalar.activation(
                            g3[h], g3p[h], mybir.ActivationFunctionType.Copy
                        )
                    for h in range(2):
                        t1[h] = apool.tile([128, BT1], bf16, tag="t1",
                                           name=f"t1{h}")
                        t1_3 = t1[h][:, :].rearrange("p (a b) -> p a b", b=QW)
                        nc.vector.tensor_tensor(
                            t1_3, bft1[h], bcast_mid(g3[h][:, :], HPC), op=MUL
                        )
                        dots[h] = apool.tile([128, HPC], bf16, tag="dots",
                                             name=f"dots{h}")
                        nc.vector.tensor_reduce(dots[h], t1_3, axis=AX, op=ADD)
                    for h in range(2):
                        e4[h] = apool.tile([128, HPC], bf16, tag="e4",
                                           name=f"e4{h}")
                        nc.scalar.activation(
                            e4[h], dots[h], mybir.ActivationFunctionType.Exp
                        )
                    for h in range(2):
                        t2[h] = apool.tile([128, BJP], bf16, tag="t2",
                                           name=f"t2{h}")
                        t2_3 = t2[h][:, :].rearrange("p (a b) -> p a b", b=HPC)
                        nc.vector.tensor_tensor(
                            t2_3, bfjp[h], bcast_mid(e4[h][:, :], JF), op=MUL
                        )
                        s4r[h] = apool.tile([128, JF], bf16, tag="s4r",
                                            name=f"s4r{h}")
                        nc.vector.tensor_reduce(s4r[h], t2_3, axis=AX, op=ADD)
                    for h in range(2):
                        grp[h] = pt.tile([HB, JF], f32, tag="tp",
                                         name=f"grp{h}")
                        nc.tensor.matmul(grp[h], e2c8, s4r[h],
                                         start=True, stop=True)
                    for h in range(2):
                        recip[h] = apool.tile([HB, 1], f32, tag="rc",
                                              name=f"rc{h}")
                        nc.vector.reciprocal(recip[h], grp[h][:, JF - 1 : JF])
                        s_sb[h] = apool.tile([HB, JF], bf16, tag="s",
                                             name=f"s{h}")
                        nc.vector.memset(s_sb[h][:, JF - 1 : JF], 1.0)
                        nc.vector.tensor_scalar_mul(
                            s_sb[h][:, 0 : JF - 1], grp[h][:, 0 : JF - 1],
                            recip[h],
                        )
                    for h in range(2):
                        sTp[h] = pt.tile([JF, HB], bf16, tag="tp",
                                         name=f"sTp{h}")
                        nc.tensor.transpose(sTp[h], s_sb[h], ident)
                        sT[h] = apool.tile([JF, HB], bf16, tag="sT",
                                           name=f"sT{h}")
                        nc.vector.tensor_copy(sT[h], sTp[h])

                # ---- t = relu(x@afin + s@sfin_aug) * |alpha|/sa ----
                ptile = {}
                for h in range(2):
                    ptile[h] = pm.tile([128, 64], f32, tag="mm", name=f"pt{h}")
                for m in range(4):
                    if l > 0:
                        for kt in range(4):
                            wsl = wb[:, (kt * 4 + m) * 128 : (kt * 4 + m + 1) * 128]
                            for h in range(2):
                                nc.tensor.matmul(
                                    ptile[h][:, m * HB : (m + 1) * HB], wsl,
                                    xbat(h, kt), start=(kt == 0), stop=False,
                                )
                    ssl = sfal[:, l * D + m * 128 : l * D + (m + 1) * 128]
                    for h in range(2):
                        nc.tensor.matmul(
                            ptile[h][:, m * HB : (m + 1) * HB], ssl,
                            sT[h], start=(l == 0), stop=True,
                        )
                tsb = {}
                for h in range(2):
                    tsb[h] = apool.tile([128, 64], bf16, tag="act",
                                        name=f"tsb{h}")
                nc.vector.tensor_scalar(
                    tsb[0], ptile[0], float(abs(al) / sa), 0.0,
                    op0=MUL, op1=mybir.AluOpType.max,
                )
                nc.scalar.activation(
                    tsb[1], ptile[1], mybir.ActivationFunctionType.Relu,
                    scale=abs(al) / sa,
                )
                for h in range(2):
                    residual_update(h, tsb[h], al >= 0)

                # ---- u = relu(x@fc0 + b0) / s0 ----
                utile = {}
                for h in range(2):
                    utile[h] = pm.tile([128, 64], f32, tag="mm", name=f"pu{h}")
                for m in range(4):
                    r0 = L + l * 8 + m
                    for h in range(2):
                        nc.tensor.matmul(
                            utile[h][:, m * HB : (m + 1) * HB], fcb_row(l, 0, m),
                            ones_at(r0, HB), start=True, stop=False,
                        )
                    for kt in range(4):
                        wsl = wb[:, 2048 + (kt * 4 + m) * 128 : 2048 + (kt * 4 + m + 1) * 128]
                        for h in range(2):
                            nc.tensor.matmul(
                                utile[h][:, m * HB : (m + 1) * HB], wsl,
                                xbat(h, kt), start=False, stop=(kt == 3),
                            )
                ub = {}
                for h in range(2):
                    ub[h] = apool.tile([128, 64], bf16, tag="act",
                                       name=f"ub{h}")
                nc.vector.tensor_scalar(
                    ub[0], utile[0], float(1.0 / s0), 0.0,
                    op0=MUL, op1=mybir.AluOpType.max,
                )
                nc.scalar.activation(
                    ub[1], utile[1], mybir.ActivationFunctionType.Relu,
                    scale=1.0 / s0,
                )

                # ---- y = (u@fc1 + b1) * alpha/s1 ----
                ytile = {}
                for h in range(2):
                    ytile[h] = pm.tile([128, 64], f32, tag="mm", name=f"py{h}")
                for m in range(4):
                    r1 = L + l * 8 + 4 + m
                    for h in range(2):
                        nc.tensor.matmul(
                            ytile[h][:, m * HB : (m + 1) * HB], fcb_row(l, 1, m),
                            ones_at(r1, HB), start=True, stop=False,
                        )
                    for kt in range(4):
                        wsl = wb[:, 4096 + (kt * 4 + m) * 128 : 4096 + (kt * 4 + m + 1) * 128]
                        for h in range(2):
                            nc.tensor.matmul(
                                ytile[h][:, m * HB : (m + 1) * HB], wsl,
                                ub[h][:, kt * HB : (kt + 1) * HB],
                                start=False, stop=(kt == 3),
                            )
                ysb = {}
                for h in range(2):
                    ysb[h] = apool.tile([128, 64], bf16, tag="act",
                                        name=f"ysb{h}")
                nc.vector.tensor_scalar(
                    ysb[0], ytile[0], float(al / s1), None, op0=MUL,
                )
                nc.scalar.activation(
                    ysb[1], ytile[1], mybir.ActivationFunctionType.Copy,
                    scale=al / s1,
                )
                for h in range(2):
                    residual_update(h, ysb[h], True)

            # ---- head: raw logits (log-softmax done host-side) ----
            for h in range(2):
                zt_ps = pt.tile([POSD, HB], f32, tag="tp", name=f"ztp{h}")
                for kt in range(4):
                    nc.tensor.matmul(
                        zt_ps, whead[:, kt * POSD : (kt + 1) * POSD],
                        xbat(h, kt), start=(kt == 0), stop=(kt == 3),
                    )
                zt = apool.tile([POSD, HB], bf16, tag="zt", name=f"zt{h}")
                nc.vector.tensor_copy(zt, zt_ps)
                lg_ps = pt.tile([HB, P], f32, tag="tp", name=f"lgp{h}")
                nc.tensor.matmul(lg_ps, zt, wpost, start=True, stop=True)
                lg = apool.tile([HB, P], f32, tag="lg", name=f"lg{h}")
                nc.vector.tensor_copy(lg, lg_ps)
                nc.sync.dma_start(out=d_out[h * HB : (h + 1) * HB, :], in_=lg)

    nc.finalize()
    return nc


def kernel(**inputs):
    inp = {k: np.asarray(v, dtype=np.float32) for k, v in inputs.items()}
    pos = _positions()
    bfeat = _prepare(inp["obs"], pos)  # (256, 81, 19)
    qk_w, qk_b, afin, sfin, cfin = _fold(inp)

    # offsets j=0 and j=6 are identical -> fold feature 6 into 0 everywhere
    qkn_w = qk_w[:, :, :7].copy()
    qkn_w[:, :, 0] += qk_w[:, :, 6]
    qkn_b = qk_b[:, :7].copy()
    qkn_b[:, 0] += qk_b[:, 6]
    sfin_f = sfin[:, :7, :].copy()
    sfin_f[:, 0, :] += sfin[:, 6, :]
    nb_f = bfeat[:, :, :6]                          # (256, 81, 6) (col0==col6)
    pos_f = bfeat[0, :, 7:]                         # (81, 12) shared

    alphas = [float(a) for a in inp["alpha"]]
    scales = []
    wf8 = np.zeros((L, 128, WF8_F), f8e3_np)
    for l in range(L):
        Aq, sa = _quant_e3(afin[l])
        F0q, s0 = _quant_e3(inp["fc0_w"][l])
        F1q, s1 = _quant_e3(inp["fc1_w"][l])
        scales.append((sa, s0, s1))
        for mat_i, Wq in enumerate((Aq, F0q, F1q)):
            t = Wq.reshape(4, 128, 4, 128)  # (kt, k, m, c)
            wf8[l, :, mat_i * 2048 : (mat_i + 1) * 2048] = (
                t.transpose(1, 0, 2, 3).reshape(128, 2048)
            )

    # sfal rows: 0..5 folded-nb sfin, 6..17 pos sfin, 18 = cfin (all * sa)
    sfal = np.zeros((JF, L * D), np.float32)
    for l in range(L):
        sa = scales[l][0]
        sfal[:JN, l * D : (l + 1) * D] = sfin_f[l, :6] * sa
        sfal[JN : JN + POSD, l * D : (l + 1) * D] = sfin[l][7:] * sa
        sfal[JF - 1, l * D : (l + 1) * D] = cfin[l] * sa

    # wqk per (l,kt) block: 18 cols = pos12 | nb6
    wqk = np.zeros((128, L * 4 * QW), np.float32)
    for l in range(L):
        for kt in range(4):
            b0 = (l * 4 + kt) * QW
            wqk[:, b0 : b0 + POSD] = qk_w[l][kt * 128 : (kt + 1) * 128, 7:]
            wqk[:, b0 + POSD : b0 + QW] = qkn_w[l][kt * 128 : (kt + 1) * 128, :6]

    wrows_flat = np.zeros((NROWS, 128), np.float32)
    for l in range(L):
        wrows_flat[l, :POSD] = qk_b[l][7:]
        wrows_flat[l, POSD:QW] = qkn_b[l][:6]
        _, s0, s1 = scales[l]
        for m in range(4):
            wrows_flat[L + l * 8 + m, :] = (
                inp["fc0_b"][l][m * 128 : (m + 1) * 128] * s0
            )
            wrows_flat[L + l * 8 + 4 + m, :] = (
                inp["fc1_b"][l][m * 128 : (m + 1) * 128] * s1
            )
    # carrier layout: row r -> partition 32*(r%3), column block r//3
    wrows = np.zeros((3, NRB * 128), np.float32)
    for r in range(NROWS):
        wrows[r % 3, (r // 3) * 128 : (r // 3 + 1) * 128] = wrows_flat[r]

    whead = (
        inp["head_w"].reshape(4, 128, POSD).transpose(1, 0, 2).reshape(128, 4 * POSD)
    )

    # layer-0 attention is x-independent: fold on host
    g0 = qk_b[0]                                    # (19,)
    dots0 = bfeat @ g0                              # (256, 81)
    e0 = np.exp(dots0)
    attn0 = e0 / e0.sum(1, keepdims=True)
    s0_all = np.einsum("bp,bpj->bj", attn0, bfeat)  # (256, 19)

    e2c8 = np.zeros((128, HB), np.float32)
    for c in range(HG):
        for b in range(HB):
            e2c8[c * HB + b, b] = 1.0
    posT = pos.reshape(P, POSD).T                   # (12, 81)

    in_maps = []
    for core in range(NCORES):
        cpk = np.zeros((128, CB_END), np.float32)
        for h in range(2):
            bsl = slice(core * NB + h * HB, core * NB + (h + 1) * HB)
            bfn = nb_f[bsl]                         # (16, 81, 6)
            for c in range(HG):
                for pp in range(HPC):
                    cell = c * HPC + pp
                    if cell >= P:
                        continue
                    rows = slice(c * HB, (c + 1) * HB)
                    # bft1: f-order pos12|nb6 (matches wqk)
                    base = C_BT1 + h * BT1 + pp * QW
                    cpk[rows, base : base + POSD] = pos_f[cell]
                    cpk[rows, base + POSD : base + QW] = bfn[:, cell, :]
                    # bfjp: f-order nb6|pos12|ones (matches sfal)
                    for f in range(JF):
                        if f < JN:
                            v = bfn[:, cell, f]
                        elif f < JF - 1:
                            v = pos_f[cell, f - JN]
                        else:
                            v = 1.0
                        cpk[rows, C_BJP + h * BJP + f * HPC + pp] = v
        cpk[:, C_E2C:C_E2T] = e2c8
        for c in range(HG):
            for b in range(HB):
                cpk[b, C_E2T + c * HB + b] = 1.0
        s0c = s0_all[core * NB : (core + 1) * NB]   # (32, 19) original order
        cpk[:JN, C_ST0 : C_ST0 + NB] = s0c[:, :6].T
        cpk[JN : JN + POSD, C_ST0 : C_ST0 + NB] = s0c[:, 7:].T
        cpk[JF - 1, C_ST0 : C_ST0 + NB] = 1.0
        cpk[:POSD, C_POST:CB_END] = posT
        in_maps.append({
            "cpk": cpk.astype(bf16_np),
            "sfal": sfal.astype(bf16_np),
            "wqk": wqk.astype(bf16_np),
            "wrows": wrows.astype(bf16_np),
            "whead": whead.astype(bf16_np),
            "wf8": wf8,
        })

    nc = _build_nc(alphas, scales)
    res = run_bass_kernel_spmd(nc, in_maps, core_ids=list(range(NCORES)))
    lg = np.concatenate([r["out"] for r in res.results], axis=0)  # (256, 81)
    lg = lg.astype(np.float32)
    lg = lg - lg.max(1, keepdims=True)
    return lg - np.log(np.exp(lg).sum(1, keepdims=True))


# revision 67
# speedup vs baseline: 1.0926x; 1.0926x over previous
# Trainium2 Bass kernel for nn_AttnModel_64098091926054.
#
# Strategy: pure data parallel over batch (256 boards -> 32 per core x 8 cores),
# with the 32 per-core boards further split into two software-pipelined halves
# of 16 so the two dependence chains interleave across PE/DVE/ACT and hide the
# cross-engine semaphore latency.
#
# Design notes:
#   - All GEMMs run "weights stationary": out^T = W^T x lands directly in the
#     x^T layout [128 dout-part, chunk*16+board], so the residual stream never
#     needs PE transposes. Each half's x is kept 4x-replicated chunk-major
#     ([128, 4kt*4c*16]); a 0-stride AP view presents it as the 8-fold
#     replication the 128-partition attention layout needs.
#   - afin/fc0/fc1 ship as fp8 (e3m4) with per-tensor power-of-2 scales folded
#     into the activation scales. Validated: max rel err ~1e-3 vs f32
#     (the gate is 2e-2); verified on hardware.
#   - Attention per half runs on 128 partitions: 81 cells padded to 8 groups
#     of 11 (pads zeroed via the bfeat constants; a 19th ones-feature row
#     produces the softmax row-sums inside the same reduce). Offsets j=0/j=6
#     are duplicates and folded host-side (6 unique nb features).
#   - Biases enter PSUM via K=1 matmuls whose [1,128] rows live on partitions
#     {0,32,64} of a carrier tile (single-partition DMAs are pathologically
#     slow; matmul operands only allow base partitions 0/32/64).
#   - Layer 0's attention is constant-folded on the host (x==0), its unused
#     afin block is not even DMA'd, and the final log-softmax runs on the
#     host, so the device tail ends at the raw-logit matmul.
#   - All DVE attention traffic is bf16 (2x DVE throughput); the residual
#     stream is bf16.

import numpy as np
import ml_dtypes

import concourse.bass as bass
import concourse.bacc as bacc
import concourse.mybir as mybir
import concourse.tile as tile
from concourse.bass_utils import run_bass_kernel_spmd
from concourse.masks import make_identity

BS, D, L, B, P, POSD, J = 9, 512, 8, 256, 81, 12, 19
NCORES = 8
NB = B // NCORES            # 32 boards per core
HB = NB // 2                # 16 boards per pipelined half
HG, HPC = 8, 11             # 81 cells -> 8 partition-groups x 11 (7 pads)
JN = 6                      # unique neighbourhood offsets (j=6 dup of j=0)
QW = POSD + JN              # 18 g-projection features: pos12 | nb6
JF = JN + POSD + 1          # 19 s-side rows: nb6 | pos12 | ones (rowsum/cfin)
OFFSETS = [(-1, 0), (-1, 1), (0, -1), (0, 0), (0, 1), (-1, -1), (-1, 0)]

f32 = mybir.dt.float32
bf16 = mybir.dt.bfloat16
f8e3 = mybir.dt.float8e3
bf16_np = ml_dtypes.bfloat16
f8e3_np = ml_dtypes.float8_e3m4

# cpk (bf16, [128, CB_END]) columns: bft1 x2 | bfjp x2 | e2c8 | sT0 | wpost
BT1 = HPC * QW                    # 198
BJP = JF * HPC                    # 209
C_BT1 = 0                         # two half-blocks back to back
C_BJP = C_BT1 + 2 * BT1           # 396
C_E2C = C_BJP + 2 * BJP           # 814
C_E2T = C_E2C + HB                # 830: (16,128) replication mask
C_ST0 = C_E2T + 128               # 958
C_POST = C_ST0 + NB               # 990
C_ROW = C_POST + P                # 1071: layer-0 fc bias rows (3 blocks)
CB_END = C_ROW + 3 * 128          # 1455

NROWS = L + L * 8                 # 8 qk_b rows + 64 fc bias rows
NRB = NROWS // 3                  # 24 row-blocks per carrier partition
WF8_F = 3 * 16 * 128              # 6144 cols: [afin|fc0|fc1] x (kt,m) tiles


def _positions():
    lin = np.linspace(0.0, 1.0, BS, dtype=np.float32)
    rs, cs = np.meshgrid(lin, lin, indexing="ij")
    zs = (rs + cs) / 2.0
    xs = np.stack([rs, cs, zs], -1).astype(np.float32)
    feats = []
    for p in [4.0 / (BS - 1), 16.0 / (BS - 1)]:
        a = (2.0 * np.pi * xs / p).astype(np.float32)
        feats.append(np.concatenate([np.cos(a), np.sin(a)], -1).astype(np.float32))
    return np.concatenate(feats, -1)  # (9, 9, 12)


def _prepare(obs, pos):
    single = obs[..., 0] - obs[..., 1]
    aug = np.pad(single, ((0, 0), (1, 1), (1, 1)))
    w = aug.shape[-1]
    outs = [aug[:, 1 + r : w - 1 + r, 1 + c : w - 1 + c] for (r, c) in OFFSETS]
    neigh = np.stack(outs, -1)
    n = obs.shape[0]
    stack = np.concatenate(
        [neigh, np.broadcast_to(pos, (n,) + pos.shape)], -1
    ).astype(np.float32)
    return stack.reshape(n, P, J)  # (B, 81, 19)


def _fold(inp):
    scale = np.float32(1.0 / np.sqrt(D))
    Wk = inp["kvb_w"][:, :, :D]
    Wv = inp["kvb_w"][:, :, D:]
    kvx_v = inp["kvx_w"][:, :, D:]
    qk_w = np.einsum("ldh,ljh->ldj", inp["q_w"], Wk) * scale      # (L,512,19)
    qk_b = np.einsum("lh,ljh->lj", inp["q_b"], Wk) * scale        # (L,19)
    afin = np.einsum("lde,leh->ldh", kvx_v, inp["fin_w"])         # (L,512,512)
    sfin = np.einsum("lje,leh->ljh", Wv, inp["fin_w"])            # (L,19,512)
    bias_v = inp["kvx_b"][:, D:] + inp["kvb_b"][:, D:]
    cfin = np.einsum("le,leh->lh", bias_v, inp["fin_w"]) + inp["fin_b"]
    return qk_w, qk_b, afin, sfin, cfin


def _quant_e3(w):
    s = np.float32(2.0 ** np.floor(np.log2(14.0 / np.abs(w).max())))
    return (w * s).astype(f8e3_np), s


def bcast_mid(ap2d, n):
    # (p, k) AP -> (p, n, k) with step-0 broadcast in the middle
    return bass.AP(
        tensor=ap2d.tensor, offset=ap2d.offset,
        ap=[ap2d.ap[0], [0, n], ap2d.ap[1]],
    )


def _build_nc(alphas, scales):
    """alphas: per-layer float; scales: (sa, s0, s1) per layer."""
    nc = bacc.Bacc("TRN2", target_bir_lowering=False, debug=False)

    d_cpk = nc.dram_tensor("cpk", [128, CB_END], bf16, kind="ExternalInput")
    d_sfal = nc.dram_tensor("sfal", [JF, L * D], bf16, kind="ExternalInput")
    d_wqk = nc.dram_tensor("wqk", [128, L * 4 * QW], bf16, kind="ExternalInput")
    d_wrows = nc.dram_tensor("wrows", [3, NRB * 128], bf16, kind="ExternalInput")
    d_whead = nc.dram_tensor("whead", [128, 4 * POSD], bf16, kind="ExternalInput")
    d_wf8 = nc.dram_tensor("wf8", [L, 128, WF8_F], f8e3, kind="ExternalInput")
    d_out = nc.dram_tensor("out", [NB, P], f32, kind="ExternalOutput")

    AX = mybir.AxisListType.X
    MUL = mybir.AluOpType.mult
    ADD = mybir.AluOpType.add

    with tile.TileContext(nc) as tc, nc.allow_low_precision(
        reason="bf16 attention path validated host-side: ~1e-3 vs 2e-2 gate"
    ):
        with (
            tc.tile_pool(name="consts", bufs=1) as consts,
            tc.tile_pool(name="wpool", bufs=8) as wpool,
            tc.tile_pool(name="ap", bufs=3) as apool,
            tc.tile_pool(name="pm", bufs=4, space="PSUM") as pm,
            tc.tile_pool(name="pt", bufs=3, space="PSUM") as pt,
        ):
            # ---- constants; DMA order tuned so layer 0 starts ASAP ----
            cpk = consts.tile([128, CB_END], bf16)
            nc.sync.dma_start(out=cpk, in_=d_cpk[:, :])
            sfal = consts.tile([JF, L * D], bf16)
            nc.sync.dma_start(out=sfal[:, 0:D], in_=d_sfal[:, 0:D])
            wf8l = [
                wpool.tile([128, WF8_F], f8e3, tag="wb", name=f"wb{i}")
                for i in range(L)
            ]
            # layer 0 never reads its afin block (x==0): ship fc0/fc1 only
            nc.sync.dma_start(out=wf8l[0][:, 2048:], in_=d_wf8[0, :, 2048:])
            # bias rows spread over partitions {0,32,64}
            wrow4 = consts.tile([65, NRB * 128], bf16)
            nc.sync.dma_start(out=wrow4[0:65:32, :], in_=d_wrows[:, :])
            wqk = consts.tile([128, L * 4 * QW], bf16)
            nc.sync.dma_start(out=wqk, in_=d_wqk[:, :])
            for l in range(1, L):
                nc.sync.dma_start(out=wf8l[l], in_=d_wf8[l, :, :])
                nc.sync.dma_start(
                    out=sfal[:, l * D : (l + 1) * D], in_=d_sfal[:, l * D : (l + 1) * D]
                )
            whead = consts.tile([128, 4 * POSD], bf16)
            nc.sync.dma_start(out=whead, in_=d_whead[:, :])

            onesr4 = consts.tile([65, 128], bf16)
            nc.vector.memset(onesr4, 1.0)
            ident = consts.tile([HB, HB], bf16)
            make_identity(nc, ident[:, :])

            bft1 = [
                cpk[:, C_BT1 + h * BT1 : C_BT1 + (h + 1) * BT1].rearrange(
                    "p (a b) -> p a b", b=QW
                )
                for h in range(2)
            ]
            bfjp = [
                cpk[:, C_BJP + h * BJP : C_BJP + (h + 1) * BJP].rearrange(
                    "p (a b) -> p a b", b=HPC
                )
                for h in range(2)
            ]
            e2c8 = cpk[:, C_E2C:C_E2T]          # (128, 16) group-sum mask
            e2c8T = cpk[:HB, C_E2T:C_E2T + 128]  # (16, 128) replication mask
            sT0 = cpk[:JF, C_ST0:C_POST]        # (19, 32) layer-0 attention
            wpost = cpk[:POSD, C_POST:C_ROW]    # (12, 81)

            def _row(r, ncols=128):
                p3 = 32 * (r % 3)
                i = r // 3
                return wrow4[p3 : p3 + 1, i * 128 : i * 128 + ncols]

            def ones_at(r, n):
                p3 = 32 * (r % 3)
                return onesr4[p3 : p3 + 1, :n]

            def fcb_row(l, stage, m):
                if l == 0:
                    r = stage * 4 + m
                    p3 = 32 * (r % 3)
                    return cpk[p3 : p3 + 1,
                               C_ROW + (r // 3) * 128 : C_ROW + (r // 3 + 1) * 128]
                return _row(L + l * 8 + stage * 4 + m)

            def fcb_ones(l, stage, m, n):
                r = (stage * 4 + m) if l == 0 else (L + l * 8 + stage * 4 + m)
                p3 = 32 * (r % 3)
                return onesr4[p3 : p3 + 1, :n]

            # per-half residual stream: plain bf16 [128, 4kt*16b]
            xch = []
            for h in range(2):
                t = apool.tile([128, 4 * HB], bf16, tag=f"xc{h}", name=f"xc_{h}")
                nc.vector.memset(t, 0.0)
                xch.append(t)

            def xbat(h, kt, n=HB):
                return xch[h][:, kt * HB : kt * HB + n]

            def residual_update(h, br_sb, positive):
                nxc = apool.tile([128, 4 * HB], bf16, tag=f"xc{h}",
                                 name=f"nxc_{h}")
                op = ADD if positive else mybir.AluOpType.subtract
                nc.vector.tensor_tensor(nxc, xch[h], br_sb, op=op)
                xch[h] = nxc

            def make_xrep8(h):
                # 8-fold replicated copy for the attention projections:
                # out col 128*kt + 16*c8 + b reads x[b] of chunk kt
                # (0-stride input APs are fine on DVE, unlike matmuls)
                r8 = apool.tile([128, 4 * 128], bf16, tag=f"xr{h}",
                                name=f"xr8_{h}")
                old = xch[h]
                in_ap = bass.AP(tensor=old.tensor, offset=old.offset,
                                ap=[old.ap[0], [HB, 4], [0, HG], [1, HB]])
                out_ap = bass.AP(tensor=r8.tensor, offset=r8.offset,
                                 ap=[r8.ap[0], [128, 4], [HB, HG], [1, HB]])
                nc.vector.tensor_copy(out_ap, in_ap)
                return r8

            for l in range(L):
                wb = wf8l[l]
                sa, s0, s1 = scales[l]
                al = alphas[l]

                # ---- attention (both halves, interleaved) ----
                if l == 0:
                    sT = [sT0[:, 0:HB], sT0[:, HB:NB]]
                else:
                    blk = (l * 4) * QW
                    g3p, g3, t1, dots, e4, t2, s4r = ({} for _ in range(7))
                    grp, recip, s_sb, sTp = {}, {}, {}, {}
                    sT = {}
                    xr8 = {}
                    for h in range(2):
                        xr8[h] = make_xrep8(h)
                    for h in range(2):
                        g3p[h] = pt.tile([128, QW], f32, tag="tp",
                                         name=f"g3p{h}")
                        for kt in range(4):
                            nc.tensor.matmul(
                                g3p[h], xr8[h][:, kt * 128 : (kt + 1) * 128],
                                wqk[:, blk + kt * QW : blk + (kt + 1) * QW],
                                start=(kt == 0), stop=False,
                            )
                        nc.tensor.matmul(
                            g3p[h], ones_at(l, 128), _row(l, QW),
                            start=False, stop=True,
                        )
                    for h in range(2):
                        g3[h] = apool.tile([128, QW], bf16, tag="g3",
                                           name=f"g3{h}")
                        nc.scalar.activation(
                            g3[h], g3p[h], mybir.ActivationFunctionType.Copy
                        )
                    for h in range(2):
                        t1[h] = apool.tile([128, BT1], bf16, tag="t1",
                                           name=f"t1{h}")
                        t1_3 = t1[h][:, :].rearrange("p (a b) -> p a b", b=QW)
                        nc.vector.tensor_tensor(
                            t1_3, bft1[h], bcast_mid(g3[h][:, :], HPC), op=MUL
                        )
                        dots[h] = apool.tile([128, HPC], bf16, tag="dots",
                                             name=f"dots{h}")
                        nc.vector.tensor_reduce(dots[h], t1_3, axis=AX, op=ADD)
                    for h in range(2):
                        e4[h] = apool.tile([128, HPC], bf16, tag="e4",
                                           name=f"e4{h}")
                        nc.scalar.activation(
                            e4[h], dots[h], mybir.ActivationFunctionType.Exp
                        )
                    for h in range(2):
                        t2[h] = apool.tile([128, BJP], bf16, tag="t2",
                                           name=f"t2{h}")
                        t2_3 = t2[h][:, :].rearrange("p (a b) -> p a b", b=HPC)
                        nc.vector.tensor_tensor(
                            t2_3, bfjp[h], bcast_mid(e4[h][:, :], JF), op=MUL
                        )
                        s4r[h] = apool.tile([128, JF], bf16, tag="s4r",
                                            name=f"s4r{h}")
                        nc.vector.tensor_reduce(s4r[h], t2_3, axis=AX, op=ADD)
                    for h in range(2):
                        grp[h] = pt.tile([HB, JF], f32, tag="tp",
                                         name=f"grp{h}")
                        nc.tensor.matmul(grp[h], e2c8, s4r[h],
                                         start=True, stop=True)
                    for h in range(2):
                        recip[h] = apool.tile([HB, 1], f32, tag="rc",
                                              name=f"rc{h}")
                        nc.vector.reciprocal(recip[h], grp[h][:, JF - 1 : JF])
                        s_sb[h] = apool.tile([HB, JF], bf16, tag="s",
                                             name=f"s{h}")
                        nc.vector.memset(s_sb[h][:, JF - 1 : JF], 1.0)
                        nc.vector.tensor_scalar_mul(
                            s_sb[h][:, 0 : JF - 1], grp[h][:, 0 : JF - 1],
                            recip[h],
                        )
                    for h in range(2):
                        sTp[h] = pt.tile([JF, HB], bf16, tag="tp",
                                         name=f"sTp{h}")
                        nc.tensor.transpose(sTp[h], s_sb[h], ident)
                        sT[h] = apool.tile([JF, HB], bf16, tag="sT",
                                           name=f"sT{h}")
                        nc.vector.tensor_copy(sT[h], sTp[h])

                # ---- t = relu(x@afin + s@sfin_aug) * |alpha|/sa ----
                ptile = {}
                for h in range(2):
                    ptile[h] = pm.tile([128, 64], f32, tag="mm", name=f"pt{h}")
                for m in range(4):
                    if l > 0:
                        for kt in range(4):
                            wsl = wb[:, (kt * 4 + m) * 128 : (kt * 4 + m + 1) * 128]
                            for h in range(2):
                                nc.tensor.matmul(
                                    ptile[h][:, m * HB : (m + 1) * HB], wsl,
                                    xbat(h, kt), start=(kt == 0), stop=False,
                                )
                    ssl = sfal[:, l * D + m * 128 : l * D + (m + 1) * 128]
                    for h in range(2):
                        nc.tensor.matmul(
                            ptile[h][:, m * HB : (m + 1) * HB], ssl,
                            sT[h], start=(l == 0), stop=True,
                        )
                tsb = {}
                for h in range(2):
                    tsb[h] = apool.tile([128, 64], bf16, tag="act",
                                        name=f"tsb{h}")
                nc.vector.tensor_scalar(
                    tsb[0], ptile[0], float(abs(al) / sa), 0.0,
                    op0=MUL, op1=mybir.AluOpType.max,
                )
                nc.scalar.activation(
                    tsb[1], ptile[1], mybir.ActivationFunctionType.Relu,
                    scale=abs(al) / sa,
                )
                for h in range(2):
                    residual_update(h, tsb[h], al >= 0)

                # ---- u = relu(x@fc0 + b0) / s0 ----
                utile = {}
                for h in range(2):
                    utile[h] = pm.tile([128, 64], f32, tag="mm", name=f"pu{h}")
                for m in range(4):
                    for h in range(2):
                        nc.tensor.matmul(
                            utile[h][:, m * HB : (m + 1) * HB], fcb_row(l, 0, m),
                            fcb_ones(l, 0, m, HB), start=True, stop=False,
                        )
                    for kt in range(4):
                        wsl = wb[:, 2048 + (kt * 4 + m) * 128 : 2048 + (kt * 4 + m + 1) * 128]
                        for h in range(2):
                            nc.tensor.matmul(
                                utile[h][:, m * HB : (m + 1) * HB], wsl,
                                xbat(h, kt), start=False, stop=(kt == 3),
                            )
                ub = {}
                for h in range(2):
                    ub[h] = apool.tile([128, 64], bf16, tag="act",
                                       name=f"ub{h}")
                nc.vector.tensor_scalar(
                    ub[0], utile[0], float(1.0 / s0), 0.0,
                    op0=MUL, op1=mybir.AluOpType.max,
                )
                nc.scalar.activation(
                    ub[1], utile[1], mybir.ActivationFunctionType.Relu,
                    scale=1.0 / s0,
                )

                # ---- y = (u@fc1 + b1) * alpha/s1 ----
                ytile = {}
                for h in range(2):
                    ytile[h] = pm.tile([128, 64], f32, tag="mm", name=f"py{h}")
                for m in range(4):
                    for h in range(2):
                        nc.tensor.matmul(
                            ytile[h][:, m * HB : (m + 1) * HB], fcb_row(l, 1, m),
                            fcb_ones(l, 1, m, HB), start=True, stop=False,
                        )
                    for kt in range(4):
                        wsl = wb[:, 4096 + (kt * 4 + m) * 128 : 4096 + (kt * 4 + m + 1) * 128]
                        for h in range(2):
                            nc.tensor.matmul(
                                ytile[h][:, m * HB : (m + 1) * HB], wsl,
                                ub[h][:, kt * HB : (kt + 1) * HB],
                                start=False, stop=(kt == 3),
                            )
                ysb = {}
                for h in range(2):
                    ysb[h] = apool.tile([128, 64], bf16, tag="act",
                                        name=f"ysb{h}")
                nc.vector.tensor_scalar(
                    ysb[0], ytile[0], float(al / s1), None, op0=MUL,
                )
                nc.scalar.activation(
                    ysb[1], ytile[1], mybir.ActivationFunctionType.Copy,
                    scale=al / s1,
                )
                for h in range(2):
                    residual_update(h, ysb[h], True)

            # ---- head: raw logits (log-softmax done host-side) ----
            for h in range(2):
                zt_ps = pt.tile([POSD, HB], f32, tag="tp", name=f"ztp{h}")
                for kt in range(4):
                    nc.tensor.matmul(
                        zt_ps, whead[:, kt * POSD : (kt + 1) * POSD],
                        xbat(h, kt), start=(kt == 0), stop=(kt == 3),
                    )
                zt = apool.tile([POSD, HB], bf16, tag="zt", name=f"zt{h}")
                nc.vector.tensor_copy(zt, zt_ps)
                lg_ps = pt.tile([HB, P], f32, tag="tp", name=f"lgp{h}")
                nc.tensor.matmul(lg_ps, zt, wpost, start=True, stop=True)
                lg = apool.tile([HB, P], f32, tag="lg", name=f"lg{h}")
                nc.vector.tensor_copy(lg, lg_ps)
                nc.sync.dma_start(out=d_out[h * HB : (h + 1) * HB, :], in_=lg)

    nc.finalize()
    return nc


def kernel(**inputs):
    inp = {k: np.asarray(v, dtype=np.float32) for k, v in inputs.items()}
    pos = _positions()
    bfeat = _prepare(inp["obs"], pos)  # (256, 81, 19)
    qk_w, qk_b, afin, sfin, cfin = _fold(inp)

    # offsets j=0 and j=6 are identical -> fold feature 6 into 0 everywhere
    qkn_w = qk_w[:, :, :7].copy()
    qkn_w[:, :, 0] += qk_w[:, :, 6]
    qkn_b = qk_b[:, :7].copy()
    qkn_b[:, 0] += qk_b[:, 6]
    sfin_f = sfin[:, :7, :].copy()
    sfin_f[:, 0, :] += sfin[:, 6, :]
    nb_f = bfeat[:, :, :6]                          # (256, 81, 6) (col0==col6)
    pos_f = bfeat[0, :, 7:]                         # (81, 12) shared

    alphas = [float(a) for a in inp["alpha"]]
    scales = []
    wf8 = np.zeros((L, 128, WF8_F), f8e3_np)
    for l in range(L):
        Aq, sa = _quant_e3(afin[l])
        F0q, s0 = _quant_e3(inp["fc0_w"][l])
        F1q, s1 = _quant_e3(inp["fc1_w"][l])
        scales.append((sa, s0, s1))
        for mat_i, Wq in enumerate((Aq, F0q, F1q)):
            t = Wq.reshape(4, 128, 4, 128)  # (kt, k, m, c)
            wf8[l, :, mat_i * 2048 : (mat_i + 1) * 2048] = (
                t.transpose(1, 0, 2, 3).reshape(128, 2048)
            )

    # sfal rows: 0..5 folded-nb sfin, 6..17 pos sfin, 18 = cfin (all * sa)
    sfal = np.zeros((JF, L * D), np.float32)
    for l in range(L):
        sa = scales[l][0]
        sfal[:JN, l * D : (l + 1) * D] = sfin_f[l, :6] * sa
        sfal[JN : JN + POSD, l * D : (l + 1) * D] = sfin[l][7:] * sa
        sfal[JF - 1, l * D : (l + 1) * D] = cfin[l] * sa

    # wqk per (l,kt) block: 18 cols = pos12 | nb6
    wqk = np.zeros((128, L * 4 * QW), np.float32)
    for l in range(L):
        for kt in range(4):
            b0 = (l * 4 + kt) * QW
            wqk[:, b0 : b0 + POSD] = qk_w[l][kt * 128 : (kt + 1) * 128, 7:]
            wqk[:, b0 + POSD : b0 + QW] = qkn_w[l][kt * 128 : (kt + 1) * 128, :6]

    wrows_flat = np.zeros((NROWS, 128), np.float32)
    for l in range(L):
        wrows_flat[l, :POSD] = qk_b[l][7:]
        wrows_flat[l, POSD:QW] = qkn_b[l][:6]
        _, s0, s1 = scales[l]
        for m in range(4):
            wrows_flat[L + l * 8 + m, :] = (
                inp["fc0_b"][l][m * 128 : (m + 1) * 128] * s0
            )
            wrows_flat[L + l * 8 + 4 + m, :] = (
                inp["fc1_b"][l][m * 128 : (m + 1) * 128] * s1
            )
    # carrier layout: row r -> partition 32*(r%3), column block r//3
    wrows = np.zeros((3, NRB * 128), np.float32)
    for r in range(NROWS):
        wrows[r % 3, (r // 3) * 128 : (r // 3 + 1) * 128] = wrows_flat[r]

    whead = (
        inp["head_w"].reshape(4, 128, POSD).transpose(1, 0, 2).reshape(128, 4 * POSD)
    )

    # layer-0 attention is x-independent: fold on host
    g0 = qk_b[0]                                    # (19,)
    dots0 = bfeat @ g0                              # (256, 81)
    e0 = np.exp(dots0)
    attn0 = e0 / e0.sum(1, keepdims=True)
    s0_all = np.einsum("bp,bpj->bj", attn0, bfeat)  # (256, 19)

    e2c8 = np.zeros((128, HB), np.float32)
    for c in range(HG):
        for b in range(HB):
            e2c8[c * HB + b, b] = 1.0
    posT = pos.reshape(P, POSD).T                   # (12, 81)

    in_maps = []
    for core in range(NCORES):
        cpk = np.zeros((128, CB_END), np.float32)
        for h in range(2):
            bsl = slice(core * NB + h * HB, core * NB + (h + 1) * HB)
            bfn = nb_f[bsl]                         # (16, 81, 6)
            for c in range(HG):
                for pp in range(HPC):
                    cell = c * HPC + pp
                    if cell >= P:
                        continue
                    rows = slice(c * HB, (c + 1) * HB)
                    # bft1: f-order pos12|nb6 (matches wqk)
                    base = C_BT1 + h * BT1 + pp * QW
                    cpk[rows, base : base + POSD] = pos_f[cell]
                    cpk[rows, base + POSD : base + QW] = bfn[:, cell, :]
                    # bfjp: f-order nb6|pos12|ones (matches sfal)
                    for f in range(JF):
                        if f < JN:
                            v = bfn[:, cell, f]
                        elif f < JF - 1:
                            v = pos_f[cell, f - JN]
                        else:
                            v = 1.0
                        cpk[rows, C_BJP + h * BJP + f * HPC + pp] = v
        cpk[:, C_E2C:C_E2T] = e2c8
        for c in range(HG):
            for b in range(HB):
                cpk[b, C_E2T + c * HB + b] = 1.0
        s0c = s0_all[core * NB : (core + 1) * NB]   # (32, 19) original order
        cpk[:JN, C_ST0 : C_ST0 + NB] = s0c[:, :6].T
        cpk[JN : JN + POSD, C_ST0 : C_ST0 + NB] = s0c[:, 7:].T
        cpk[JF - 1, C_ST0 : C_ST0 + NB] = 1.0
        cpk[:POSD, C_POST:C_ROW] = posT
        for r in range(8):
            cpk[32 * (r % 3), C_ROW + (r // 3) * 128 : C_ROW + (r // 3 + 1) * 128] = (
                wrows_flat[L + r]
            )
        in_maps.append({
            "cpk": cpk.astype(bf16_np),
            "sfal": sfal.astype(bf16_np),
            "wqk": wqk.astype(bf16_np),
            "wrows": wrows.astype(bf16_np),
            "whead": whead.astype(bf16_np),
            "wf8": wf8,
        })

    nc = _build_nc(alphas, scales)
    res = run_bass_kernel_spmd(nc, in_maps, core_ids=list(range(NCORES)))
    lg = np.concatenate([r["out"] for r in res.results], axis=0)  # (256, 81)
    lg = lg.astype(np.float32)
    lg = lg - lg.max(1, keepdims=True)
    return lg - np.log(np.exp(lg).sum(1, keepdims=True))
